# revision 1
# baseline (speedup 1.0000x reference)
"""Trainium2 Bass kernel for AdaptiveScaledDotProductAttention.

Sharding: DP=4 over batch x TP=2 over heads (8 NeuronCores).
Core c handles batch c//2, head-group g=c%2 (heads 8g..8g+7).
Each core projects q/k/v/s for its 8 heads over the full sequence,
runs attention, then the pair exchanges attention outputs (AllToAll)
so core g output-projects rows [512g, 512g+512) with all 16 heads.

On-chip layout: feature-major ("transposed") activations so every
matmul contraction sits on the partition dim without runtime
transposes beyond one PE-transpose pass over the raw inputs.
Softmax denominators ride along as an extra ones-column of V; the
per-query language logit rides as an extra matmul row.
"""

import numpy as np

H, DK, DV, DM = 16, 64, 64, 1024
B, N = 4, 1024
SCALE = float(1.0 / np.sqrt(DK))
NCORES = 8
HLOC = 8  # heads per core
HDLOC = HLOC * DK  # 512
NH = N // 2  # 512, output rows per core

_CACHE = {}
DEBUG_TAPS = False
K_ITER = 1  # >1: loop whole kernel in-graph (timing only)


def _build(with_biases, k_iter=1):
    import concourse.bass as bass
    import concourse.tile as tile
    from concourse import bacc, mybir
    from concourse.masks import make_identity

    f32 = mybir.dt.float32
    f32r = mybir.dt.float32r
    bf16 = mybir.dt.bfloat16
    Exp = mybir.ActivationFunctionType.Exp
    Copy = mybir.ActivationFunctionType.Copy

    nc = bacc.Bacc("TRN2", target_bir_lowering=False, debug=False,
                   num_devices=NCORES)

    def din(name, shape):
        return nc.dram_tensor(name, shape, f32, kind="ExternalInput").ap()

    xq = din("queries", [N, DM])
    xk = din("keys", [N, DM])
    xv = din("values", [N, DM])
    xs = din("signals", [N, DM])
    wq = din("wq", [DM, HDLOC])
    wk = din("wk", [DM, HDLOC])
    wv = din("wv", [DM, HDLOC])
    ws = din("ws", [DM, HDLOC])
    wo = din("wo", [H * DV, DM // 2])
    bq = din("bq", [1, HDLOC])
    bk = din("bk", [1, HDLOC])
    bv = din("bv", [1, HDLOC])
    bs = din("bs", [1, HDLOC])
    bo = din("bo", [1, DM // 2])
    out = nc.dram_tensor("out", [N, DM // 2], f32, kind="ExternalOutput").ap()
    dbg = {}
    if DEBUG_TAPS:
        for nm, shp, dt_ in (
                ("d_qT", [DK, HLOC, N], "bf16"), ("d_kT", [DK, HLOC, N], "bf16"),
                ("d_sT", [DK, HLOC, N], "bf16"),
                ("d_vaug", [128, 8, HLOC, DV + 1], "bf16"),
                ("d_E", [128, 8, 512], "bf16"),
                ("d_u", [DK, HLOC, 512], "f32"),
                ("d_rA", [HLOC, 512], "f32"), ("d_rB", [HLOC, 512], "f32"),
                ("d_st2", [2, 512], "f32"), ("d_lo", [DK, DK + 2], "bf16"),
                ("d_p", [DK, 512], "bf16"),
                ("d_outT", [DK, HLOC, N], "bf16")):
            dd = mybir.dt.bfloat16 if dt_ == "bf16" else f32
            dbg[nm] = nc.dram_tensor(nm, shp, dd, kind="ExternalOutput").ap()

    RG = [[0, 1], [2, 3], [4, 5], [6, 7]]

    from contextlib import ExitStack
    with ExitStack() as top:
        tc = top.enter_context(tile.TileContext(nc))

        persist = top.enter_context(tc.tile_pool(name="persist", bufs=1))
        # feature-major projection outputs, one base-0 plane per head
        qTp = persist.tile([DK, HLOC, N], bf16)
        kTp = persist.tile([DK, HLOC, N], bf16)
        sTp = persist.tile([DK, HLOC, N], bf16)
        # token-major V with a ones column per head: [k-part, kchunk, head, 65]
        vaug = persist.tile([128, 8, HLOC, DV + 1], bf16)
        # attention outputs (feature-major planes), all n
        outT = persist.tile([DK, HLOC, N], bf16)
        identity = persist.tile([128, 128], bf16)
        langones = persist.tile([DK, DK + 2], bf16)  # col 65 = 1
        sel = persist.tile([HLOC, HLOC * DK], bf16)  # row-select for bcast mms
        onesrow = persist.tile([1, 512], bf16)

        make_identity(nc, identity)
        nc.vector.memset(vaug[:, :, :, DV:DV + 1], 1.0)
        nc.vector.memset(langones[:, :], 0.0)
        nc.vector.memset(langones[:, DK + 1:DK + 2], 1.0)
        # sel[p, a, b] = 1 where a == p  (row-select matrices for bcast mms)
        nc.gpsimd.memset(sel[:, :], 0.0)
        nc.gpsimd.affine_select(
            out=sel.rearrange("p (a b) -> p a b", a=HLOC),
            in_=sel.rearrange("p (a b) -> p a b", a=HLOC),
            compare_op=mybir.AluOpType.not_equal,
            fill=1.0,
            base=0,
            pattern=[[-1, HLOC], [0, DV]],
            channel_multiplier=1)
        nc.vector.memset(onesrow[:, :], 1.0)

        for _it in range(k_iter):
            if with_biases:
                bias_sb = {}
                for nm, ap in (("bq", bq), ("bk", bk), ("bv", bv), ("bs", bs),
                               ("bo", bo)):
                    tf = persist.tile([1, ap.shape[1]], f32, tag=f"biasf_{nm}")
                    nc.sync.dma_start(out=tf, in_=ap)
                    t = persist.tile([1, ap.shape[1]], bf16, tag=f"bias_{nm}")
                    nc.vector.tensor_copy(out=t, in_=tf)
                    bias_sb[nm] = t

            # ---------------- Phase A: transposes + projections ----------------
            with ExitStack() as pa:
                wpool = pa.enter_context(tc.tile_pool(name="wpool", bufs=1))
                wstage = pa.enter_context(tc.tile_pool(name="wstage", bufs=2))
                w_sb = {}
                for nm, ap in (("wq", wq), ("wk", wk), ("wv", wv), ("ws", ws)):
                    tf = wstage.tile([128, 8, HDLOC], f32, tag="wstage")
                    nc.sync.dma_start(out=tf,
                                      in_=ap.rearrange("(j p) c -> p j c", p=128))
                    t = wpool.tile([128, 8, HDLOC], bf16, tag=f"w_{nm}")
                    nc.vector.tensor_copy(out=t, in_=tf)
                    w_sb[nm] = t

                xrow = pa.enter_context(tc.tile_pool(name="xrow", bufs=6))
                xtp = pa.enter_context(tc.tile_pool(name="xtpool", bufs=2))
                t_psum = pa.enter_context(
                    tc.tile_pool(name="t_psum", bufs=2, space="PSUM"))
                p_psum = pa.enter_context(
                    tc.tile_pool(name="p_psum", bufs=4, space="PSUM"))

                for tname, xin in (("q", xq), ("k", xk), ("v", xv), ("s", xs)):
                    for nch in range(2):
                        rows = []
                        for nt in range(4):
                            rf = xrow.tile([128, DM], f32, tag="xrowf")
                            nc.sync.dma_start(
                                out=rf, in_=xin[nch * 512 + nt * 128:
                                                nch * 512 + (nt + 1) * 128, :])
                            r = xrow.tile([128, DM], bf16, tag="xrow")
                            if nt % 2 == 0:
                                nc.vector.tensor_copy(out=r, in_=rf)
                            else:
                                nc.scalar.activation(r, rf, Copy)
                            rows.append(r)
                        xt = xtp.tile([128, 8, 512], bf16, tag="xt")
                        for j in range(8):
                            ps = t_psum.tile([128, 512], bf16, tag="tps")
                            for nt in range(4):
                                nc.tensor.transpose(
                                    ps[:, nt * 128:(nt + 1) * 128],
                                    rows[nt][:, j * 128:(j + 1) * 128],
                                    identity)
                            if j % 2 == 0:
                                nc.vector.tensor_copy(out=xt[:, j, :], in_=ps)
                            else:
                                nc.scalar.activation(xt[:, j, :], ps, Copy)

                        if tname in ("q", "k", "s"):
                            wsb = w_sb["w" + tname]
                            dst = {"q": qTp, "k": kTp, "s": sTp}[tname]
                            for ht in range(4):
                                ps = p_psum.tile([128, 512], f32, tag="pps")
                                for j in range(8):
                                    nc.tensor.matmul(
                                        ps,
                                        wsb[:, j, ht * 128:(ht + 1) * 128],
                                        xt[:, j, :],
                                        start=(j == 0), stop=(j == 7))
                                if with_biases:
                                    nc.tensor.matmul(
                                        ps,
                                        bias_sb["b" + tname][:, ht * 128:(ht + 1) * 128],
                                        onesrow[:, :512],
                                        start=False, stop=True)
                                nc.vector.tensor_copy(
                                    out=dst[:, 2 * ht, nch * 512:(nch + 1) * 512],
                                    in_=ps[0:64, :])
                                nc.scalar.activation(
                                    dst[:, 2 * ht + 1, nch * 512:(nch + 1) * 512],
                                    ps[64:128, :], Copy)
                        else:  # values: token-major
                            for nt in range(4):
                                kc = nch * 4 + nt
                                ps = p_psum.tile([128, 512], f32, tag="pps")
                                for j in range(8):
                                    nc.tensor.matmul(
                                        ps,
                                        xt[:, j, nt * 128:(nt + 1) * 128],
                                        w_sb["wv"][:, j, :],
                                        start=(j == 0), stop=(j == 7))
                                if with_biases:
                                    nc.tensor.matmul(
                                        ps,
                                        onesrow[:, :128],
                                        bias_sb["bv"],
                                        start=False, stop=True)
                                nc.vector.tensor_copy(
                                    out=vaug[:, kc, :, 0:DV],
                                    in_=ps.rearrange("p (h d) -> p h d", h=HLOC))

            if DEBUG_TAPS:
                nc.sync.dma_start(out=dbg["d_qT"], in_=qTp)
                nc.sync.dma_start(out=dbg["d_kT"], in_=kTp)
                nc.sync.dma_start(out=dbg["d_sT"], in_=sTp)
                nc.sync.dma_start(out=dbg["d_vaug"], in_=vaug)

            # ---------------- Phase B: attention ----------------
            with ExitStack() as pb:
                sc_psum = pb.enter_context(
                    tc.tile_pool(name="sc_psum", bufs=2, space="PSUM"))
                av_psum = pb.enter_context(
                    tc.tile_pool(name="av_psum", bufs=2, space="PSUM"))
                b_psum = pb.enter_context(
                    tc.tile_pool(name="b_psum", bufs=1, space="PSUM"))
                epool = pb.enter_context(tc.tile_pool(name="epool", bufs=2))
                ppool = pb.enter_context(tc.tile_pool(name="ppool", bufs=3))
                upool = pb.enter_context(tc.tile_pool(name="upool", bufs=2))
                rpool = pb.enter_context(tc.tile_pool(name="rpool", bufs=2))
                bspool = pb.enter_context(tc.tile_pool(name="bspool", bufs=3))
                tpool = pb.enter_context(tc.tile_pool(name="tpool", bufs=4))

                for qc in range(2):
                    qs = slice(qc * 512, (qc + 1) * 512)
                    rA = rpool.tile([HLOC, 512], f32, tag="rA")
                    rB = rpool.tile([HLOC, 512], f32, tag="rB")
                    u_sb = upool.tile([DK, HLOC, 512], f32, tag="usb")
                    avs = []
                    for h in range(HLOC):
                        p = ppool.tile([DK, 512], bf16, tag="p")
                        nc.vector.tensor_mul(p, qTp[:, h, qs], sTp[:, h, qs])
                        E = epool.tile([128, 8, 512], bf16, tag="E")
                        for kc2 in range(4):
                            sc = sc_psum.tile([128, 1024], f32, tag="sc")
                            for half in range(2):
                                c = 2 * kc2 + half
                                nc.tensor.matmul(
                                    sc[:, half * 512:(half + 1) * 512],
                                    kTp[:, h, c * 128:(c + 1) * 128],
                                    qTp[:, h, qs])
                            nc.scalar.activation(
                                E[:, 2 * kc2:2 * kc2 + 2, :].rearrange(
                                    "p a b -> p (a b)"),
                                sc, Exp, scale=SCALE)
                        if DEBUG_TAPS and qc == 0 and h == 0:
                            nc.sync.dma_start(out=dbg["d_E"], in_=E)
                        av = av_psum.tile([128, 512], f32, tag="av")
                        # lang matmul first: start=True initializes rows 0..65
                        # (cols 0..64 of langones are zero), row 65 = lang logits
                        nc.tensor.matmul(
                            av[0:DV + 2, :],
                            langones,
                            p,
                            start=True, stop=False)
                        for c in range(8):
                            nc.tensor.matmul(
                                av[0:DV + 1, :],
                                vaug[:, c, h, :],
                                E[:, c, :],
                                start=False, stop=(c == 7))
                        st2 = tpool.tile([2, 512], f32, tag="st2")
                        nc.vector.tensor_copy(out=st2, in_=av[DV:DV + 2, :])
                        if DEBUG_TAPS and qc == 0 and h == 0:
                            nc.sync.dma_start(out=dbg["d_st2"], in_=st2)
                            nc.sync.dma_start(out=dbg["d_lo"], in_=langones)
                            nc.sync.dma_start(out=dbg["d_p"], in_=p)
                        nc.sync.dma_start(out=rA[h:h + 1, :], in_=st2[0:1, :])
                        nc.sync.dma_start(out=rB[h:h + 1, :], in_=st2[1:2, :])
                        nc.scalar.activation(u_sb[:, h, :], av[0:DV, :], Copy)
                        avs.append(av)

                    # batched softmax scalar path for all 8 heads
                    el = rpool.tile([HLOC, 512], f32, tag="el")
                    nc.scalar.activation(el, rB, Exp, scale=SCALE)
                    dn = rpool.tile([HLOC, 512], f32, tag="dn")
                    nc.vector.tensor_add(dn, rA, el)  # denom
                    rc = rpool.tile([HLOC, 512], f32, tag="rcp")
                    nc.vector.reciprocal(rc, dn)      # 1/denom
                    w2f = rpool.tile([HLOC, 512], f32, tag="w2f")
                    nc.vector.tensor_mul(w2f, rc, el)  # e_lang/denom
                    rAb = rpool.tile([HLOC, 512], bf16, tag="rAb")
                    rBb = rpool.tile([HLOC, 512], bf16, tag="rBb")
                    nc.vector.tensor_copy(out=rAb, in_=rc)
                    nc.vector.tensor_copy(out=rBb, in_=w2f)
                    if DEBUG_TAPS and qc == 0:
                        nc.sync.dma_start(out=dbg["d_rA"], in_=rc)
                        nc.sync.dma_start(out=dbg["d_rB"], in_=w2f)
                        nc.sync.dma_start(out=dbg["d_u"], in_=u_sb)

                    for h in range(HLOC):
                        b = b_psum.tile([DV, 1024], f32, tag="b")
                        nc.tensor.matmul(
                            b[:, 0:512],
                            sel[:, DV * h:DV * h + DV],
                            rAb)
                        nc.tensor.matmul(
                            b[:, 512:1024],
                            sel[:, DV * h:DV * h + DV],
                            rBb)
                        bsb = bspool.tile([DV, 1024], f32, tag="bsb")
                        nc.vector.tensor_copy(out=bsb, in_=b)
                        t1 = tpool.tile([DV, 512], f32, tag="t1")
                        nc.vector.tensor_mul(t1, u_sb[:, h, :], bsb[:, 0:512])
                        t2 = tpool.tile([DV, 512], f32, tag="t2")
                        nc.vector.tensor_mul(t2, sTp[:, h, qs], bsb[:, 512:1024])
                        nc.vector.tensor_add(outT[:, h, qs], t1, t2)

            if DEBUG_TAPS:
                nc.sync.dma_start(out=dbg["d_outT"], in_=outT)

            # ---------------- Phase C: exchange + output projection -----------
            with ExitStack() as pc:
                dpool = pc.enter_context(
                    tc.tile_pool(name="dpool", bufs=1, space="DRAM"))
                cpool = pc.enter_context(tc.tile_pool(name="cpool", bufs=1))
                o_psum = pc.enter_context(
                    tc.tile_pool(name="o_psum", bufs=4, space="PSUM"))
                o_stage = pc.enter_context(tc.tile_pool(name="o_stage", bufs=3))

                ex_in = dpool.tile([HDLOC, N], bf16)
                ex_out = dpool.tile([2 * HDLOC, N], bf16)
                for h in range(HLOC):
                    nc.sync.dma_start(
                        out=ex_in[h * DK:(h + 1) * DK, :],
                        in_=outT[:, h, :])
                nc.gpsimd.collective_compute(
                    "AllGather", mybir.AluOpType.bypass,
                    replica_groups=RG,
                    ins=[ex_in[:].opt()], outs=[ex_out[:].opt()])

                oT = cpool.tile([128, 8, N], bf16)
                nc.sync.dma_start(
                    out=oT, in_=ex_out.rearrange("(c p) n -> p c n", p=128))
                wo_f = cpool.tile([128, 8, DM // 2], f32)
                nc.sync.dma_start(
                    out=wo_f, in_=wo.rearrange("(c p) m -> p c m", p=128))
                wo_sb = cpool.tile([128, 8, DM // 2], bf16)
                nc.vector.tensor_copy(out=wo_sb, in_=wo_f)

                for nt in range(8):
                    ps = o_psum.tile([128, 512], f32, tag="ops")
                    for c in range(8):
                        nc.tensor.matmul(
                            ps,
                            oT[:, c, nt * 128:(nt + 1) * 128],
                            wo_sb[:, c, :],
                            start=(c == 0), stop=(c == 7))
                    if with_biases:
                        nc.tensor.matmul(
                            ps,
                            onesrow[:, :128],
                            bias_sb["bo"],
                            start=False, stop=True)
                    ostage = o_stage.tile([128, 512], f32, tag="ostage")
                    if nt % 2 == 0:
                        nc.vector.tensor_copy(out=ostage, in_=ps)
                    else:
                        nc.scalar.activation(ostage, ps, Copy)
                    nc.sync.dma_start(
                        out=out[nt * 128:(nt + 1) * 128, :],
                        in_=ostage)

    nc.compile()
    return nc


def _get_nc(with_biases):
    key = ("nc", with_biases, K_ITER)
    if key not in _CACHE:
        _CACHE[key] = _build(with_biases, K_ITER)
    return _CACHE[key]


def kernel(queries, keys, values, language_signals,
           Wq, b_q, Wk, b_k, Wv, b_v, Ws, b_s, Wo, b_o):
    from concourse.bass_utils import run_bass_kernel_spmd

    with_biases = any(
        np.any(np.asarray(b)) for b in (b_q, b_k, b_v, b_s, b_o))
    nc = _get_nc(with_biases)

    f = np.float32
    in_maps = []
    for core in range(NCORES):
        b, g = core // 2, core % 2
        hs = slice(HDLOC * g, HDLOC * (g + 1))
        in_maps.append({
            "queries": np.ascontiguousarray(queries[b], dtype=f),
            "keys": np.ascontiguousarray(keys[b], dtype=f),
            "values": np.ascontiguousarray(values[b], dtype=f),
            "signals": np.ascontiguousarray(language_signals[b], dtype=f),
            "wq": np.ascontiguousarray(Wq[:, hs], dtype=f),
            "wk": np.ascontiguousarray(Wk[:, hs], dtype=f),
            "wv": np.ascontiguousarray(Wv[:, hs], dtype=f),
            "ws": np.ascontiguousarray(Ws[:, hs], dtype=f),
            "wo": np.ascontiguousarray(Wo[:, NH * g:NH * (g + 1)], dtype=f),
            "bq": np.ascontiguousarray(b_q[hs], dtype=f).reshape(1, -1),
            "bk": np.ascontiguousarray(b_k[hs], dtype=f).reshape(1, -1),
            "bv": np.ascontiguousarray(b_v[hs], dtype=f).reshape(1, -1),
            "bs": np.ascontiguousarray(b_s[hs], dtype=f).reshape(1, -1),
            "bo": np.ascontiguousarray(
                b_o[NH * g:NH * (g + 1)], dtype=f).reshape(1, -1),
        })
    _CACHE["last_in_maps"] = in_maps
    res = run_bass_kernel_spmd(nc, in_maps, list(range(NCORES))).results
    full = np.empty((B, N, DM), np.float32)
    for core in range(NCORES):
        b, g = core // 2, core % 2
        full[b, :, NH * g:NH * (g + 1)] = res[core]["out"]
    return full

